# revision 3
# baseline (speedup 1.0000x reference)
"""Trainium2 Bass kernel for nn_DiffusionGraphConv (gnn_message_passing).

Reference computation (B=64, N=1024, D=128=64+64, O=128, 2 supports,
2 diffusion steps):
    x0 = concat(inputs, state)                      # [B, N, D]
    y1 = S0 x0 ; z2 = S0 y1 ; y3 = S1 y1 ; z4 = S1 y3
    xs = [x0, y1, 2 z2 - x0, y3, 2 z4 - y1]
    out = concat_d(xs) @ W + bias                   # [B*N, O]

Polynomial refactor: fold the +-/2x into weight blocks
    Wa = W0 - W2, Wb = W1 - W4, Wc = 2 W2, Wd = W3, We = 2 W4
then commute the (cheap, K=128) feature projections with the node-space
supports and hoist the batch-independent support polynomials:
    M1 = S0, M2 = S0^2, M3 = S1 S0, M4 = S1^2 S0      (precomputed once)
    out = (x0 Wa + bias) + sum_k M_k (x0 W'_k),  W' = [Wb, Wc, Wd, We]

Sharding: data-parallel over batch, 8 batches per NeuronCore; supports,
their polynomials and the weights stay SBUF-resident.

Mixed precision: the apply phase (the PE-roofline term, ~4.3G MACs/core)
runs a configurable subset of (k, 256-row contraction block) units in
fp8e4m3 with perf_mode=DoubleRow (2 fp8 MACs/cell/cycle), the rest in
bf16. All operands carry power-of-2 scale factors folded in on the host
(A*512, Q*16, region weights *8192) so every PSUM accumulation shares one
scale and every drain is a plain copy; the host divides by 8192 and adds
the bias at the end. The fp8 unit set is sized so the exact offline
simulation of the quantization noise stays well under the 2e-2 gate
(f=0.375 of the contraction -> 1.49e-2 vs all-bf16's 4.9e-3).

Per-core schedule:
    pre:   UT = T1 T1 ; A2T = T0s T0 ; A3T = T0s T1 ; A4T = T0s UT
           (T0s = 512 S0^T etc.; drains emit bf16 or paired-fp8 per unit)
    per rep:
      Q:     Qb..Qe = x0 W'_k per (nt, h): one stationary x0t slice feeds
             4 matmuls into 4 PSUM banks; drains emit bf16 / paired-fp8
      apply: out[it, f] = PSUM( sum_units AkT MM Qk ) + x0 Wa'  (DR fp8
             units use [128,2,*] paired APs; DVE copy + DMA out)
"""
import sys

if "/opt/trn_rl_repo" not in sys.path:
    sys.path.insert(0, "/opt/trn_rl_repo")

import numpy as np
import ml_dtypes

import concourse.bass as bass
import concourse.mybir as mybir
from concourse import bacc, tile
from concourse.bass_utils import run_bass_kernel_spmd

N_CORES = 8
B = 64
BL = B // N_CORES          # local batches per core
N = 1024                   # nodes
D = 128                    # input_size (64 input + 64 hidden)
O = 128                    # output_size
NT = N // 128              # node partition tiles
BF16 = mybir.dt.bfloat16
F8 = mybir.dt.float8e4
F32 = mybir.dt.float32
DR = mybir.MatmulPerfMode.DoubleRow

SS = 512.0                 # stationary scale (power of 2)
SQ = 16.0                  # Q scale (power of 2)

# (k, p) units computed in fp8-DoubleRow; k indexes [M1,M2,M3,M4] terms,
# p the 256-row contraction block. Chosen by offline error sim.
FP8_UNITS = frozenset({(2, 0), (2, 1), (2, 2), (2, 3),
                       (0, 0), (0, 1), (0, 2), (0, 3)})

_CACHE = {}


def _fmt8(k, p):
    return (k, p) in FP8_UNITS


def _build(reps=1):
    nc = bacc.Bacc("TRN2", target_bir_lowering=False, debug=False,
                   num_devices=N_CORES)
    s0t_d = nc.dram_tensor("s0t", [N, N], BF16, kind="ExternalInput").ap()
    s0n_d = nc.dram_tensor("s0n", [N, N], BF16, kind="ExternalInput").ap()
    s1t_d = nc.dram_tensor("s1t", [N, N], BF16, kind="ExternalInput").ap()
    s1n_d = nc.dram_tensor("s1n", [N, N], BF16, kind="ExternalInput").ap()
    s0ts_d = nc.dram_tensor("s0ts", [N, N], BF16, kind="ExternalInput").ap()
    s0t8_d = nc.dram_tensor("s0t8", [4 * 128, 2048], F8,
                            kind="ExternalInput").ap()
    x0t_d = nc.dram_tensor("x0t", [BL * D, N], BF16, kind="ExternalInput").ap()
    wf_d = nc.dram_tensor("wf", [5 * D, O], BF16, kind="ExternalInput").ap()
    out_d = nc.dram_tensor("out", [N, BL, O], BF16, kind="ExternalOutput").ap()

    with tile.TileContext(nc) as tc:
        with (
            tc.tile_pool(name="main", bufs=1) as mp,
            tc.tile_pool(name="outp", bufs=4) as op,
            tc.tile_pool(name="psb", bufs=8, space="PSUM") as pb,
        ):
            # ---- persistent SBUF residents ----
            # DMA emission order = consumption order (precompute first).
            s1n = []   # buffers later reused for Qc (bf16)
            s1t = []   # later reused for Qe (bf16)
            s0n = []   # later reused for Qd fp8 pair tiles (first 4)
            s0t = []   # later reused for Qb (bf16 jt<4, fp8 pairs jt>=4)
            for j in range(NT):
                t = mp.tile([128, N], BF16, tag=f"qc{j}", name=f"s1n{j}")
                nc.sync.dma_start(out=t[:], in_=s1n_d[j * 128:(j + 1) * 128, :])
                s1n.append(t)
                t = mp.tile([128, N], BF16, tag=f"qe{j}", name=f"s1t{j}")
                nc.sync.dma_start(out=t[:], in_=s1t_d[j * 128:(j + 1) * 128, :])
                s1t.append(t)
            for j in range(NT):
                t = mp.tile([128, N], BF16, tag=f"qdx{j}", name=f"s0n{j}")
                nc.sync.dma_start(out=t[:], in_=s0n_d[j * 128:(j + 1) * 128, :])
                s0n.append(t)
                t = mp.tile([128, N], BF16, tag=f"qbx{j}", name=f"s0t{j}")
                nc.sync.dma_start(out=t[:], in_=s0t_d[j * 128:(j + 1) * 128, :])
                s0t.append(t)
            x0t0 = []
            for b in range(BL):
                t = mp.tile([128, N], BF16, tag=f"x0t{b}_0", name=f"x0t{b}_p")
                nc.sync.dma_start(out=t[:], in_=x0t_d[b * 128:(b + 1) * 128, :])
                x0t0.append(t)
            w = []
            for k in range(5):
                t = mp.tile([128, O], BF16, tag=f"w{k}", name=f"w{k}")
                nc.sync.dma_start(out=t[:], in_=wf_d[k * 128:(k + 1) * 128, :])
                w.append(t)

            # k0 apply stationaries straight from HBM (bf16 and fp8 paired)
            s0ts = {}
            for p in range(4):
                if _fmt8(0, p):
                    t = mp.tile([128, 2, 1024], F8, tag=f"s0t8_{p}",
                                name=f"s0t8_{p}")
                    nc.sync.dma_start(
                        out=t[:], in_=s0t8_d[p * 128:(p + 1) * 128, :])
                    s0ts[p] = t
                else:
                    pair = []
                    for h in range(2):
                        jt = 2 * p + h
                        t = mp.tile([128, N], BF16, tag=f"s0ts{jt}",
                                    name=f"s0ts{jt}")
                        nc.sync.dma_start(
                            out=t[:], in_=s0ts_d[jt * 128:(jt + 1) * 128, :])
                        pair.append(t)
                    s0ts[p] = pair

            ci = 0

            def pcopy(dst, src):
                # alternate DVE / ACT for PSUM->SBUF moves
                nonlocal ci
                if ci % 2 == 0:
                    nc.vector.tensor_copy(dst, src)
                else:
                    nc.scalar.copy(dst, src)
                ci += 1

            # ---- precompute support polynomials (batch-independent) ----
            # UT = (S1^2)^T ; A2T = 512 (S0^2)^T ; A3T = 512 (S1 S0)^T ;
            # A4T = 512 (S1^2 S0)^T  (s0n is uploaded pre-scaled by 512)
            ut = [mp.tile([128, N], BF16, tag=f"ut{j}", name=f"ut{j}")
                  for j in range(NT)]

            def stat_tiles(k, prefix):
                # per-pair stationary storage for a precomputed mat
                st = {}
                for p in range(4):
                    if _fmt8(k, p):
                        st[p] = mp.tile([128, 2, 1024], F8,
                                        tag=f"{prefix}8_{p}",
                                        name=f"{prefix}8_{p}")
                    else:
                        st[p] = [mp.tile([128, N], BF16,
                                         tag=f"{prefix}{2 * p + h}",
                                         name=f"{prefix}{2 * p + h}")
                                 for h in range(2)]
                return st

            a2t = stat_tiles(1, "a2t")
            a3t = stat_tiles(2, "a3t")
            a4t = stat_tiles(3, "a4t")

            def drain_stat(st, k, it, f, ps):
                p, h = it // 2, it % 2
                if _fmt8(k, p):
                    pcopy(st[p][:, h, f * 512:(f + 1) * 512], ps[:])
                else:
                    pcopy(st[p][h][:, f * 512:(f + 1) * 512], ps[:])

            for dst, kk, lhs, rhs in ((ut, None, s1n, s1t),
                                      (a2t, 1, s0n, s0t),
                                      (a3t, 2, s0n, s1t),
                                      (a4t, 3, s0n, ut)):
                for it in range(NT):
                    for f in range(2):
                        ps = pb.tile([128, 512], F32, tag="big")
                        for jt in range(NT):
                            nc.tensor.matmul(
                                ps[:],
                                lhs[jt][:, it * 128:(it + 1) * 128],
                                rhs[jt][:, f * 512:(f + 1) * 512],
                                start=(jt == 0), stop=(jt == NT - 1),
                            )
                        if kk is None:
                            pcopy(dst[it][:, f * 512:(f + 1) * 512], ps[:])
                        else:
                            drain_stat(dst, kk, it, f, ps)

            mats = [s0ts, a2t, a3t, a4t]

            for rep in range(reps):
                # ---- x0 T-layout reload (rep 0 preloaded); its last reader
                # is this rep's apply phase, so rep r+1's DMA hides under
                # rep r's tail.
                if rep == 0:
                    x0t = x0t0
                else:
                    x0t = []
                    for b in range(BL):
                        t = mp.tile([128, N], BF16,
                                    tag=f"x0t{b}_{rep % 2}",
                                    name=f"x0t{b}_{rep}")
                        nc.sync.dma_start(
                            out=t[:], in_=x0t_d[b * 128:(b + 1) * 128, :])
                        x0t.append(t)

                # Q tiles: per (k, pair) bf16 [128, N] x2 or fp8 [128,2,1024]
                qs = []
                for k, pre in ((0, "qb"), (1, "qc"), (2, "qd"), (3, "qe")):
                    qk = {}
                    for p in range(4):
                        if _fmt8(k, p):
                            qk[p] = mp.tile(
                                [128, 2, 1024], F8,
                                tag=f"qdx{2 * p}" if k == 2
                                    else f"qbx{2 * p}",
                                name=f"{pre}8_{p}_{rep}")
                        else:
                            qk[p] = [mp.tile(
                                [128, BL * O], BF16,
                                tag=f"{pre}{2 * p + h}" if k in (1, 3)
                                    else f"qbx{2 * p + h}",
                                name=f"{pre}{2 * p + h}_{rep}")
                                for h in range(2)]
                    qs.append(qk)

                def drain_q(k, nt, hsl, ps):
                    p, h = nt // 2, nt % 2
                    if _fmt8(k, p):
                        dst = qs[k][p][:, h, hsl]
                    else:
                        dst = qs[k][p][h][:, hsl]
                    if k % 2 == 0:
                        nc.scalar.copy(dst, ps[:])
                    else:
                        nc.vector.tensor_copy(dst, ps[:])

                # ---- Q phase: Qk = x0 W'_k (k=b..e), layout [n,(b,o)] ----
                # One stationary x0t slice feeds 4 matmuls into 4 PSUM
                # banks; 2:2 drain split across ACT / DVE.
                for nt in range(NT):
                    for h in range(2):
                        ps4 = [pb.tile([128, 512], F32, tag="big",
                                       name=f"q{k}_{rep}_{nt}_{h}")
                               for k in range(4)]
                        for q, bb in enumerate(range(4 * h, 4 * h + 4)):
                            stat = x0t[bb][:, nt * 128:(nt + 1) * 128]
                            for k in range(4):
                                nc.tensor.matmul(
                                    ps4[k][:, q * 128:(q + 1) * 128],
                                    stat, w[k + 1][:],
                                    start=True, stop=True)
                        hsl = slice(h * 512, (h + 1) * 512)
                        for k in range(4):
                            drain_q(k, nt, hsl, ps4[k])

                # ---- apply: out[it, f] = sum_k M_k Qk + x0 Wa' ----
                for it in range(NT):
                    ps2 = [pb.tile([128, 512], F32, tag="big",
                                   name=f"fin_{rep}_{it}_{f}")
                           for f in range(2)]
                    isl = slice(it * 128, (it + 1) * 128)
                    first = True
                    for k in range(4):
                        st = mats[k]
                        qk = qs[k]
                        for p in range(4):
                            if _fmt8(k, p):
                                lhsT = st[p][:, :, isl]
                                for f in range(2):
                                    nc.tensor.matmul(
                                        ps2[f][:], lhsT,
                                        qk[p][:, :, f * 512:(f + 1) * 512],
                                        start=first, stop=False,
                                        perf_mode=DR,
                                        skip_group_check=True)
                                first = False
                            else:
                                for h in range(2):
                                    stat = st[p][h][:, isl]
                                    for f in range(2):
                                        nc.tensor.matmul(
                                            ps2[f][:], stat,
                                            qk[p][h][:, f * 512:(f + 1) * 512],
                                            start=first, stop=False,
                                            skip_group_check=True)
                                    first = False
                    # x0 Wa' region-adds last: start=False accumulates into
                    # the already-written banks
                    for f in range(2):
                        for q, bb in enumerate(range(4 * f, 4 * f + 4)):
                            nc.tensor.matmul(
                                ps2[f][:, q * 128:(q + 1) * 128],
                                x0t[bb][:, isl],
                                w[0][:], start=False, stop=(q == 3),
                                skip_group_check=True)
                    for f in range(2):
                        ot = op.tile([128, 512], BF16, tag="out")
                        nc.vector.tensor_copy(ot[:], ps2[f][:])
                        nc.sync.dma_start(
                            out=out_d[it * 128:(it + 1) * 128,
                                      4 * f:4 * f + 4, :],
                            in_=ot[:])
    nc.compile()
    _dedup_ldweights(nc)
    return nc


def _dedup_ldweights(nc):
    """Drop InstLdweights that reload the exact weights already resident in
    the PE array (same physical AP as the previous load, no intervening
    weight-state change, no waits/updates to preserve)."""
    removed = 0
    for fn in nc.m.functions:
        for blk in fn.blocks:
            insts = blk.instructions
            keep = []
            last_ap = None
            for inst in insts:
                if str(inst.engine) == "EngineType.PE":
                    if isinstance(inst, mybir.InstLdweights):
                        ap = repr(inst.ins[0])
                        if (ap == last_ap and not inst.has_wait()
                                and not inst.has_update()):
                            removed += 1
                            continue
                        last_ap = ap
                    elif isinstance(inst, mybir.InstMatmult):
                        if inst.ldweights is not False:
                            last_ap = None  # self-loading MM clobbers state
                    else:
                        last_ap = None  # drain/branch/etc: be conservative
                keep.append(inst)
            if removed and len(keep) != len(insts):
                blk.instructions = keep
    return removed


def _prep_inputs(supports, inputs, state, weight, biases):
    supports = np.asarray(supports, dtype=np.float32)
    inputs = np.asarray(inputs, dtype=np.float32)
    state = np.asarray(state, dtype=np.float32)
    weight = np.asarray(weight, dtype=np.float32)

    bf = ml_dtypes.bfloat16
    f8 = ml_dtypes.float8_e4m3
    s0n = (supports[0] * SS).astype(bf)            # pre-scaled lhs
    s0t = np.ascontiguousarray(supports[0].T).astype(bf)
    s1n = supports[1].astype(bf)
    s1t = np.ascontiguousarray(supports[1].T).astype(bf)
    s0ts = np.ascontiguousarray(supports[0].T * SS).astype(bf)
    # paired-fp8 layout for DoubleRow: [p*128+kk, i2*1024+i] =
    #   512 * S0^T[256p + 128 i2 + kk, i]
    t8 = (supports[0].T * SS).astype(f8)
    s0t8 = np.ascontiguousarray(
        t8.reshape(4, 2, 128, 1024).transpose(0, 2, 1, 3).reshape(512, 2048))

    x0 = np.concatenate(
        [inputs.reshape(B, N, D // 2), state.reshape(B, N, D // 2)], axis=2)
    x0t = np.ascontiguousarray(x0.transpose(0, 2, 1))      # [B, D, N]
    x0t_bf = x0t.astype(bf)

    W = weight.reshape(5, D, O)
    wf = np.concatenate([
        (W[0] - W[2]) * (SS * SQ),   # Wa'
        (W[1] - W[4]) * SQ,          # Wb'
        2.0 * W[2] * SQ,             # Wc'
        W[3] * SQ,                   # Wd'
        2.0 * W[4] * SQ,             # We'
    ], axis=0).astype(bf)

    in_maps = []
    for c in range(N_CORES):
        bsl = slice(c * BL, (c + 1) * BL)
        in_maps.append({
            "s0t": s0t,
            "s0n": s0n,
            "s1t": s1t,
            "s1n": s1n,
            "s0ts": s0ts,
            "s0t8": s0t8,
            "x0t": np.ascontiguousarray(x0t_bf[bsl]).reshape(BL * D, N),
            "wf": wf,
        })
    return in_maps


def _get_runner(reps=1):
    """Build the jitted SPMD executor once (mirrors
    bass2jax.run_bass_via_pjrt) so repeated calls don't re-trace."""
    if ("runner", reps) in _CACHE:
        return _CACHE[("runner", reps)]
    import jax
    from jax.sharding import Mesh, PartitionSpec, NamedSharding
    from concourse import bass2jax
    import concourse.mybir as mb

    try:
        jax.config.update("jax_compilation_cache_dir", "/tmp/jax_cache")
        jax.config.update("jax_persistent_cache_min_compile_time_secs", 1.0)
    except Exception:
        pass

    if ("nc", reps) not in _CACHE:
        _CACHE[("nc", reps)] = _build(reps=reps)
    nc = _CACHE[("nc", reps)]
    bass2jax.install_neuronx_cc_hook()

    part_name = nc.partition_id_tensor.name if nc.partition_id_tensor else None
    in_names, out_names, out_avals, zero_outs = [], [], [], []
    for alloc in nc.m.functions[0].allocations:
        if not isinstance(alloc, mb.MemoryLocationSet):
            continue
        name = alloc.memorylocations[0].name
        if alloc.kind == "ExternalInput":
            if name != part_name:
                in_names.append(name)
        elif alloc.kind == "ExternalOutput":
            out_names.append(name)
            shape = tuple(alloc.tensor_shape)
            dtype = mb.dt.np(alloc.dtype)
            out_avals.append(jax.core.ShapedArray(shape, dtype))
            zero_outs.append(np.zeros(shape, dtype))
    n_params = len(in_names)
    all_names = in_names + out_names
    if part_name is not None:
        all_names = all_names + [part_name]

    def _body(*args):
        operands = list(args)
        if part_name is not None:
            operands.append(bass2jax.partition_id_tensor())
        outs = bass2jax._bass_exec_p.bind(
            *operands,
            out_avals=tuple(out_avals),
            in_names=tuple(all_names),
            out_names=tuple(out_names),
            lowering_input_output_aliases=(),
            sim_require_finite=True,
            sim_require_nnan=True,
            nc=nc,
        )
        return tuple(outs)

    devices = jax.devices()[:N_CORES]
    mesh = Mesh(np.asarray(devices), ("core",))
    from jax.experimental.shard_map import shard_map
    n_outs = len(out_names)
    donate = tuple(range(n_params, n_params + n_outs))
    sharded = jax.jit(
        shard_map(_body, mesh=mesh,
                  in_specs=(PartitionSpec("core"),) * (n_params + n_outs),
                  out_specs=(PartitionSpec("core"),) * n_outs,
                  check_rep=False),
        donate_argnums=donate, keep_unused=True)
    sh = NamedSharding(mesh, PartitionSpec("core"))

    runner = {
        "fn": sharded, "in_names": in_names, "out_names": out_names,
        "zero_outs": zero_outs, "sharding": sh, "mesh": mesh,
    }
    _CACHE[("runner", reps)] = runner
    return runner


def _run(in_maps, device_inputs=None, reps=1):
    """Execute on the 8 cores; returns list of per-core output dicts."""
    import jax
    r = _get_runner(reps)
    if device_inputs is None:
        device_inputs = _put_inputs(in_maps, reps)
    zeros = [
        jax.device_put(
            np.zeros((N_CORES * z.shape[0], *z.shape[1:]), z.dtype),
            r["sharding"])
        for z in r["zero_outs"]
    ]
    out_arrs = r["fn"](*device_inputs, *zeros)
    outs = [np.asarray(a) for a in out_arrs]
    return [
        {name: outs[i].reshape(N_CORES, *r["zero_outs"][i].shape)[c]
         for i, name in enumerate(r["out_names"])}
        for c in range(N_CORES)
    ]


def _put_inputs(in_maps, reps=1):
    import jax
    r = _get_runner(reps)
    return [
        jax.device_put(
            np.concatenate([np.asarray(in_maps[c][n]) for c in range(N_CORES)],
                           axis=0), r["sharding"])
        for n in r["in_names"]
    ]


def kernel(supports, inputs, state, weight, biases, output_size=O, **_):
    assert int(output_size) == O
    biases = np.asarray(biases, dtype=np.float32)
    in_maps = _prep_inputs(supports, inputs, state, weight, biases)
    res = _run(in_maps)
    # per-core out: [N, BL, O] -> full [B, N*O]; descale + bias on host
    outs = np.stack([res[c]["out"] for c in range(N_CORES)]).astype(np.float32)
    outs = outs * (1.0 / (SS * SQ)) + biases[None, None, None, :]
    out = outs.transpose(0, 2, 1, 3).reshape(B, N * O)
    return np.ascontiguousarray(out)


if __name__ == "__main__":
    rng = np.random.default_rng(0)
    sup = rng.standard_normal((2, N, N)).astype(np.float32) / np.sqrt(N)
    inp = rng.standard_normal((B, N * 64)).astype(np.float32)
    st = rng.standard_normal((B, N * 64)).astype(np.float32)
    wt = rng.standard_normal((5 * D, O)).astype(np.float32) * 0.05
    bs = np.zeros((O,), np.float32)
    out = kernel(sup, inp, st, wt, bs, O)
    print("out", out.shape, out.dtype, float(np.abs(out).max()))
